# revision 16
# baseline (speedup 1.0000x reference)
"""Trainium2 Bass kernel for soft decision-tree histogram binning.

Math (per row n of x[N=2048, F=8], cut_points[F, D=3], T=0.1):
    W = [1, 2, 3, 4];  cs = sort(cut_points, axis=1)
    b[f] = cumsum([0, -cs[f,0], -cs[f,1], -cs[f,2]])
    h[n,f,:] = x[n,f] * W + b[f]
    bins[n,f,:] = softmax(h / T)              # [N, F, 4]
    out[n] = kron_f bins[n,f,:]               # [N, 4^8 = 65536]

Strategy: pure data-parallel over 8 NeuronCores (256 rows each). Output is
512 MB fp32 -> the kernel is HBM-write-bound (~64 MB/core at ~358 GB/s).
Per 128-row tile we compute unnormalized exps e[128, 8, 4] (per-feature
max-subtracted, temperature folded into the ACT exp scale), normalize once
at the end via 1/prod(group sums), and build the Kronecker product as
  A[16]  = (e0 * 1/P) (x) e1
  B[4096] = e2 (x) e3 (x) e4 (x) e5 (x) e6 (x) e7
  out[:, a*4096:(a+1)*4096] = B * A[:, a]     (16 DVE tensor_scalar ops)
Each 4096-wide chunk is a 2 MiB DMA straight to HBM.
"""

import sys

import numpy as np

for _p in ("/opt/trn_rl_repo",):
    if _p not in sys.path:
        sys.path.insert(0, _p)

import concourse.bass as bass
import concourse.tile as tile
from concourse import mybir
from concourse.bass_utils import run_bass_kernel_spmd

TEMPERATURE = 0.1
N, F, NB = 2048, 8, 4  # NB = D+1 bins per feature
NCORES = 8
NLOC = N // NCORES  # 256 rows per core
OUT_COLS = NB**F  # 65536
ROW_TILE = 128
A_COLS = NB * NB  # 16   = kron(e0, e1)
B_COLS = NB**6  # 4096 = kron(e2..e7)
SUPER = 2  # a-values batched into one contiguous SBUF super-chunk per DMA
OBUF_BUFS = 4
f32 = mybir.dt.float32

# test.py can flip these to profile; harness just calls kernel().
RUN_KWARGS: dict = {}
LAST_RESULTS = None

_cache: dict = {}


def _build_nc() -> bass.Bass:
    nc = bass.Bass()
    x_d = nc.declare_dram_parameter("x", [NLOC, F], f32, isOutput=False)
    # consts row layout: [0:4] = W, [4:36] = b[f, j] row-major; replicated x128
    c_d = nc.declare_dram_parameter("consts", [128, NB + F * NB], f32, isOutput=False)
    o_d = nc.declare_dram_parameter("out", [NLOC, OUT_COLS], f32, isOutput=True)

    MUL = mybir.AluOpType.mult
    ADD = mybir.AluOpType.add
    SUB = mybir.AluOpType.subtract
    AX = mybir.AxisListType.X

    with tile.TileContext(nc) as tc:
        with (
            tc.tile_pool(name="singles", bufs=1) as singles,
            tc.tile_pool(name="work", bufs=2) as work,
            tc.tile_pool(name="big", bufs=2) as big,
            tc.tile_pool(name="obuf", bufs=OBUF_BUFS) as obufs,
        ):
            NT = NLOC // ROW_TILE  # row-tiles per core (2)

            cst = singles.tile([128, NB + F * NB], f32)
            nc.sync.dma_start(out=cst, in_=c_d[:])
            # Bounce constants through a DVE copy: TensorTensor's ISA struct
            # has a single sync-wait slot, so TT ops must not wait on a DMA
            # lane and the DVE semaphore at the same time.
            cstS = singles.tile([128, NB + F * NB], f32)
            nc.vector.tensor_copy(cstS[:], cst[:])
            cW = cstS[:, 0:NB]  # [128, 4]
            cB = cstS[:, NB:].rearrange("p (f j) -> p f j", j=NB)  # [128, 8, 4]

            # Fused softmax prep for BOTH row-tiles at once ([128, NT, F, NB]):
            # the small ops are fixed-overhead dominated, so halving their
            # count shortens the critical path to the first output DMA.
            xt = singles.tile([128, NT, F], f32)
            nc.sync.dma_start(out=xt[:, 0, :], in_=x_d[0:ROW_TILE, :])
            nc.sync.dma_start(out=xt[:, 1, :], in_=x_d[ROW_TILE : 2 * ROW_TILE, :])

            xe = singles.tile([128, NT, F, NB], f32)
            nc.vector.tensor_copy(
                xe[:], xt.unsqueeze(3).to_broadcast([128, NT, F, NB])
            )
            # h = x[:, f] * W[j] + b[f, j]
            h = singles.tile([128, NT, F, NB], f32)
            nc.vector.tensor_tensor(
                h[:],
                xe[:],
                cW.unsqueeze(1).unsqueeze(1).to_broadcast([128, NT, F, NB]),
                op=MUL,
            )
            nc.vector.tensor_tensor(
                h[:], h[:], cB.unsqueeze(1).to_broadcast([128, NT, F, NB]), op=ADD
            )
            # per-(row, feature) max over the 4 bins, for exp stability
            m = singles.tile([128, NT, F], f32)
            nc.vector.reduce_max(m, h[:], axis=AX)
            hm = singles.tile([128, NT, F, NB], f32)
            nc.vector.tensor_tensor(
                hm[:], h[:], m.unsqueeze(3).to_broadcast([128, NT, F, NB]), op=SUB
            )
            # e = exp((h - m) / T)  (scale folds in the temperature)
            e2t = singles.tile([128, NT, F, NB], f32)
            nc.scalar.activation(
                e2t[:], hm[:], mybir.ActivationFunctionType.Exp, scale=1.0 / TEMPERATURE
            )
            # group sums -> product over features -> reciprocal
            s = singles.tile([128, NT, F], f32)
            nc.vector.reduce_sum(s, e2t[:], axis=AX)
            p4 = singles.tile([128, NT, 4], f32)
            nc.vector.tensor_tensor(p4[:], s[:, :, 0:4], s[:, :, 4:8], op=MUL)
            p2 = singles.tile([128, NT, 2], f32)
            nc.vector.tensor_tensor(p2[:], p4[:, :, 0:2], p4[:, :, 2:4], op=MUL)
            p1 = singles.tile([128, NT], f32)
            nc.vector.tensor_tensor(
                p1[:], p2[:, :, 0:1].squeeze(2), p2[:, :, 1:2].squeeze(2), op=MUL
            )
            rP = singles.tile([128, NT], f32)
            nc.vector.reciprocal(rP[:], p1[:])

            for t in range(NT):
                r0 = t * ROW_TILE
                e = e2t[:, t, :, :]  # [128, F, NB]

                # A[16] = (e0 * rP) (x) e1   -- one fused scalar_tensor_tensor
                A16 = work.tile([128, NB, NB], f32)
                nc.vector.scalar_tensor_tensor(
                    A16[:],
                    e[:, 0, :].unsqueeze(2).to_broadcast([128, NB, NB]),
                    rP[:, t : t + 1],
                    e[:, 1, :].unsqueeze(1).to_broadcast([128, NB, NB]),
                    op0=MUL,
                    op1=MUL,
                )
                A16f = A16.rearrange("p a b -> p (a b)")

                # B[4096] = e2 (x) e3 (x) e4 (x) e5 (x) e6 (x) e7
                t23 = work.tile([128, NB, NB], f32)
                nc.vector.tensor_tensor(
                    t23[:],
                    e[:, 2, :].unsqueeze(2).to_broadcast([128, NB, NB]),
                    e[:, 3, :].unsqueeze(1).to_broadcast([128, NB, NB]),
                    op=MUL,
                )
                t45 = work.tile([128, NB, NB], f32)
                nc.vector.tensor_tensor(
                    t45[:],
                    e[:, 4, :].unsqueeze(2).to_broadcast([128, NB, NB]),
                    e[:, 5, :].unsqueeze(1).to_broadcast([128, NB, NB]),
                    op=MUL,
                )
                t67 = work.tile([128, NB, NB], f32)
                nc.vector.tensor_tensor(
                    t67[:],
                    e[:, 6, :].unsqueeze(2).to_broadcast([128, NB, NB]),
                    e[:, 7, :].unsqueeze(1).to_broadcast([128, NB, NB]),
                    op=MUL,
                )
                t23f = t23.rearrange("p a b -> p (a b)")
                t45f = t45.rearrange("p a b -> p (a b)")
                t67f = t67.rearrange("p a b -> p (a b)")
                t2345 = work.tile([128, 16, 16], f32)
                nc.vector.tensor_tensor(
                    t2345[:],
                    t23f.unsqueeze(2).to_broadcast([128, 16, 16]),
                    t45f.unsqueeze(1).to_broadcast([128, 16, 16]),
                    op=MUL,
                )
                t2345f = t2345.rearrange("p a b -> p (a b)")
                B4096 = big.tile([128, 256, 16], f32)
                nc.vector.tensor_tensor(
                    B4096[:],
                    t2345f.unsqueeze(2).to_broadcast([128, 256, 16]),
                    t67f.unsqueeze(1).to_broadcast([128, 256, 16]),
                    op=MUL,
                )
                B4096f = B4096.rearrange("p a b -> p (a b)")

                # expansion: out chunk a = B * A[:, a]; SUPER chunks of 4096
                # share one contiguous SBUF tile so each store is one big DMA
                for a0 in range(0, A_COLS, SUPER):
                    ob = obufs.tile([128, SUPER * B_COLS], f32)
                    for j in range(SUPER):
                        nc.vector.tensor_scalar_mul(
                            ob[:, j * B_COLS : (j + 1) * B_COLS],
                            B4096f,
                            A16f[:, a0 + j : a0 + j + 1],
                        )
                    # alternate between the two HWDGE rings (SP and ACT)
                    dma_eng = nc.sync if (a0 // SUPER) % 2 == 0 else nc.scalar
                    dma_eng.dma_start(
                        out=o_d[
                            r0 : r0 + ROW_TILE,
                            a0 * B_COLS : (a0 + SUPER) * B_COLS,
                        ],
                        in_=ob[:],
                    )
    return nc


def _split_multi_waits(nc: bass.Bass) -> None:
    """Walrus' CoreV3 compute-ISA structs carry a single sync-wait slot, but
    Tile (with optimize_sems disabled) can attach 2+ waits to one compute
    instruction. Hoist all but one wait onto dedicated same-engine NoOps
    inserted right before the instruction — the engine blocks on each in
    program order, so semantics are identical."""
    skip = {"InstEventSemaphore", "InstNoOp"}
    counter = [0]
    for fn in nc.m.functions:
        for bb in fn.blocks:
            insts = bb.instructions
            i = 0
            while i < len(insts):
                ins = insts[i]
                si = getattr(ins, "sync_info", None)
                if (
                    type(ins).__name__ not in skip
                    and si is not None
                    and si.on_wait
                    and len(si.on_wait) > 1
                ):
                    extra, keep = si.on_wait[:-1], si.on_wait[-1:]
                    for w in extra:
                        counter[0] += 1
                        nop = mybir.InstEventSemaphore(
                            name=f"I-waitsplit-{counter[0]}",
                            engine=ins.engine,
                            bass_nofuse=True,
                            sync_info=mybir.SyncInfo(on_wait=[w], on_update=[]),
                            bass_scheduled_tick=ins.bass_scheduled_tick,
                            bass_scheduled_proc=ins.bass_scheduled_proc,
                            bass_scheduled_scope=ins.bass_scheduled_scope,
                            debug=ins.debug,
                        )
                        insts.insert(i, nop)
                        i += 1
                    si.on_wait = keep
                i += 1


def _get_nc() -> bass.Bass:
    if "nc" not in _cache:
        nc = _build_nc()
        _split_multi_waits(nc)
        _cache["nc"] = nc
    return _cache["nc"]


def _host_consts(cut_points: np.ndarray) -> np.ndarray:
    cs = np.sort(np.asarray(cut_points, dtype=np.float32), axis=1)  # [F, D]
    b = np.concatenate([np.zeros((F, 1), np.float32), -cs], axis=1)
    b = np.cumsum(b, axis=1, dtype=np.float32)  # [F, 4]
    W = np.linspace(1.0, float(NB), NB).astype(np.float32)  # [1, 2, 3, 4]
    row = np.concatenate([W, b.reshape(-1)]).astype(np.float32)  # [36]
    return np.ascontiguousarray(np.broadcast_to(row, (128, row.size)))


def kernel(x: np.ndarray, cut_points: np.ndarray) -> np.ndarray:
    global LAST_RESULTS
    x = np.ascontiguousarray(x, dtype=np.float32)
    consts = _host_consts(cut_points)
    nc = _get_nc()
    in_maps = [
        {"x": x[i * NLOC : (i + 1) * NLOC], "consts": consts} for i in range(NCORES)
    ]
    res = run_bass_kernel_spmd(nc, in_maps, list(range(NCORES)), **RUN_KWARGS)
    LAST_RESULTS = res
    return np.concatenate([r["out"] for r in res.results], axis=0)
